# revision 2
# baseline (speedup 1.0000x reference)
"""Trainium2 Bass kernel v5 for nn_BidPrefix (segment_reduce).

Per row r (B=65536, S=512): cp[k] = prod(x[r, 0:k]).  Outputs:
    survival = cp[bid]
    rate     = cp[mp] - cp[mp+1] = cp[mp] * (1 - x[mp])   (EPS when mp == 0)

v5: segment-granular pipeline (4 segments x 16 tiles), L = 32 blocks.
  - Host sorts rows by w = max(bid, mp) descending; per-tile widths W_t
    (multiple of 32).  Each segment's x is packed in a 32-plane halving
    layout, split into two DMA chunks with INTERLEAVED planes
    (a = planes 0-7+16-23, b = planes 8-15+24-31) so s1 runs per chunk:
      s1_a = a_lo*a_hi, s1_b = b_lo*b_hi -> b1;  s2..s5 halve b1 -> p32.
    6 fp16 2x TensorTensor per segment on DVE.
  - Act: ln(p32 + 1e-38) per segment -> lnp32 fp16 [128, tot32].
  - Prefix selection without per-tile ops:
      thrG[p,t] = o32_t + floor(k/32)   (host, fp16, global block coords)
      thrGB     = thrG bcast over blocks (Pool grouped stride-0 copies)
      iog       = global block iota      (Pool iota + copy)
      mask      = (thrGB > iog)          (DVE fp16 2x)
      mk        = mask * lnp32           (Pool mult, per segment)
      sc        = segment cumsum         (DVE tensor_tensor_scan, f32 out)
      g         = sc at static tile-end columns (Pool ap_gather)
      braw[t]   = g[t] - g[t-1]          (DVE sub per segment)
  - Act: exp on braw; DVE postfix with host-fused straddle scalars.
  - Segment processing order [2, 0, 1, 3]: cheap segment first (fast
    pipeline start) and cheapest last (short tail).
Straddle products over the <=31 leading elements of the boundary block and
x[mp] ship as fp16 aux scalars computed on host during sharding.
"""

import numpy as np

import concourse.bacc as bacc
from concourse.bass import AP
from concourse.hw_specs import get_activation_tables
import concourse.mybir as mybir
from concourse.tile import TileContext
from concourse.bass_utils import run_bass_kernel_spmd

f32 = mybir.dt.float32
f16 = mybir.dt.float16
i32 = mybir.dt.int32
i16 = mybir.dt.int16
Alu = mybir.AluOpType
Act = mybir.ActivationFunctionType

N_CORES = 8
B, S = 65536, 512
ROWS = B // N_CORES          # 8192 rows per core
NT = ROWS // 128             # 64 tiles of 128 rows
L = 64                       # block size
NSEG = 4
TPS = NT // NSEG             # 16 tiles per segment
SEG_ORDER = [2, 0, 1, 3]
EPS = 1e-7


def seg_geometry(widths):
    nb = [w // L for w in widths]
    o32 = np.concatenate([[0], np.cumsum(nb)]).astype(int)
    segw = [int(o32[(s + 1) * TPS] - o32[s * TPS]) for s in range(NSEG)]
    return nb, o32, segw


def build_bass(widths):
    nt = NT
    assert len(widths) == nt
    nb, o32, segw = seg_geometry(widths)
    tot32 = int(o32[-1])
    P = tot32 * L
    nc = bacc.Bacc()

    xpk = nc.dram_tensor("xpk", [128, P], f16, kind="ExternalInput")
    aux16 = nc.dram_tensor("aux16", [ROWS, 4], f16, kind="ExternalInput")
    thr16 = nc.dram_tensor("thr16", [ROWS, 2], f16, kind="ExternalInput")
    gidx = nc.dram_tensor("gidx", [128, NSEG], i16, kind="ExternalInput")
    surv_out = nc.dram_tensor("survival", [ROWS, 1], f32, kind="ExternalOutput")
    rate_out = nc.dram_tensor("rate_last", [ROWS, 1], f32, kind="ExternalOutput")

    aux_v = aux16.rearrange("(p t) c -> p t c", t=nt)
    thr_v = thr16.rearrange("(p t) c -> p t c", t=nt)
    so_v = surv_out.rearrange("(p t) c -> p (t c)", t=nt)
    ro_v = rate_out.rearrange("(p t) c -> p (t c)", t=nt)

    # xpk element offsets: segments in tile order; each segment is
    # [chunk_a | chunk_b], chunk = 16 planes * NBs elements
    seg_off = np.concatenate([[0], np.cumsum([w * L for w in segw])]).astype(int)
    sb_max = max(segw)

    groups = []
    t = 0
    while t < nt:
        t2 = t
        while t2 < nt and nb[t2] == nb[t]:
            t2 += 1
        groups.append((t, t2))
        t = t2

    with TileContext(nc) as tc:
        with (
            tc.tile_pool(name="xp", bufs=4) as xpool,
            tc.tile_pool(name="c1", bufs=2) as c1pool,
            tc.tile_pool(name="c2", bufs=2) as c2pool,
            tc.tile_pool(name="pk", bufs=1) as pk,
        ):
            # thr first: it gates mask prep and must beat x to the DMA queue
            thrt = pk.tile([128, nt, 2], f16, tag="thrt")
            nc.sync.dma_start(out=thrt[:], in_=thr_v)
            auxt = pk.tile([128, nt, 4], f16, tag="auxt")
            nc.scalar.dma_start(out=auxt[:], in_=aux_v)
            git = pk.tile([128, NSEG], i16, tag="git")
            nc.scalar.dma_start(out=git[:], in_=gidx[:, :])

            names = list(get_activation_tables(nc.m.arch).keys())
            nc.scalar.add_instruction(mybir.InstLoadActFuncSet(
                name=nc.get_next_instruction_name(),
                act_func_set_id=names.index("natural_log_exp_and_others"),
                ins=[], outs=[]))

            # Pool prep: iota, iog, ones, thrGB broadcast
            it = pk.tile([128, tot32], i32, tag="it")
            nc.gpsimd.iota(it[:], pattern=[[1, tot32]], base=0,
                           channel_multiplier=0)
            iog = pk.tile([128, tot32], f16, tag="iog")
            nc.vector.tensor_copy(out=iog[:], in_=it[:])
            ones = pk.tile([128, tot32], f16, tag="ones")
            nc.gpsimd.memset(ones[:], 1.0)

            thrgb = pk.tile([128, 2, tot32], f16, tag="thrgb")
            for (ga, gb) in groups:
                nbg = nb[ga]
                gsz = gb - ga
                src = thrt[:, ga:gb, :]
                bsrc = AP(src.tensor, src.offset,
                          [src.ap[0], [1, 2], [2, gsz], [0, nbg]])
                dst = thrgb[:, :, int(o32[ga]):int(o32[gb])]
                bdst = AP(dst.tensor, dst.offset,
                          [dst.ap[0], [tot32, 2], [nbg, gsz], [1, nbg]])
                nc.vector.tensor_copy(out=bdst, in_=bsrc)

            # masks on DVE (runs in the pre-first-chunk window)
            maskb = pk.tile([128, tot32], f16, tag="maskb")
            nc.vector.tensor_tensor(out=maskb[:], in0=thrgb[:, 0, :],
                                    in1=iog[:], op=Alu.is_gt)
            maskm = pk.tile([128, tot32], f16, tag="maskm")
            nc.vector.tensor_tensor(out=maskm[:], in0=thrgb[:, 1, :],
                                    in1=iog[:], op=Alu.is_gt)

            lnbias = pk.tile([128, 1], f32, tag="lnbias")
            nc.vector.memset(lnbias[:], 1e-38)

            p32 = pk.tile([128, tot32], f16, tag="p32")
            lnp32 = pk.tile([128, tot32], f16, tag="lnp32")
            mkb = pk.tile([128, tot32], f16, tag="mkb")
            mkm = pk.tile([128, tot32], f16, tag="mkm")
            scb = pk.tile([128, tot32], f32, tag="scb")
            scm = pk.tile([128, tot32], f32, tag="scm")
            braw = pk.tile([128, 2 * nt], f32, tag="braw")
            gbb = pk.tile([128, NSEG, TPS + 1], f32, tag="gbb")
            gbm = pk.tile([128, NSEG, TPS + 1], f32, tag="gbm")
            nc.vector.memset(gbb[:, :, 0], 0.0)
            nc.vector.memset(gbm[:, :, 0], 0.0)

            for si in SEG_ORDER:
                NBs = segw[si]
                E = int(seg_off[si])          # xpk element base of segment
                g0 = int(o32[si * TPS])
                g1 = int(o32[(si + 1) * TPS])
                H = (L // 4) * NBs            # half-chunk width

                xa = xpool.tile([128, sb_max * (L // 2)], f16, tag="xa")
                nc.sync.dma_start(out=xa[:, :2 * H], in_=xpk[:, E:E + 2 * H])
                xb_ = xpool.tile([128, sb_max * (L // 2)], f16, tag="xb")
                nc.sync.dma_start(out=xb_[:, :2 * H],
                                  in_=xpk[:, E + 2 * H:E + 4 * H])

                b1 = c1pool.tile([128, sb_max * 32], f16, tag="b1")
                nc.vector.tensor_tensor(out=b1[:, :H], in0=xa[:, :H],
                                        in1=xa[:, H:2 * H], op=Alu.mult)
                nc.vector.tensor_tensor(out=b1[:, H:2 * H], in0=xb_[:, :H],
                                        in1=xb_[:, H:2 * H], op=Alu.mult)
                b2 = c2pool.tile([128, sb_max * 16], f16, tag="b2")
                nc.vector.tensor_tensor(out=b2[:, :H], in0=b1[:, :H],
                                        in1=b1[:, H:2 * H], op=Alu.mult)
                b3 = c2pool.tile([128, sb_max * 8], f16, tag="b3")
                nc.vector.tensor_tensor(out=b3[:, :H // 2], in0=b2[:, :H // 2],
                                        in1=b2[:, H // 2:H], op=Alu.mult)
                b4 = c2pool.tile([128, sb_max * 4], f16, tag="b4")
                nc.vector.tensor_tensor(out=b4[:, :H // 4], in0=b3[:, :H // 4],
                                        in1=b3[:, H // 4:H // 2], op=Alu.mult)
                b5 = c2pool.tile([128, sb_max * 2], f16, tag="b5")
                nc.vector.tensor_tensor(out=b5[:, :H // 8], in0=b4[:, :H // 8],
                                        in1=b4[:, H // 8:H // 4], op=Alu.mult)
                nc.vector.tensor_tensor(out=p32[:, g0:g1], in0=b5[:, :H // 16],
                                        in1=b5[:, H // 16:H // 8], op=Alu.mult)
                nc.scalar.activation(out=lnp32[:, g0:g1], in_=p32[:, g0:g1],
                                     func=Act.Ln, bias=lnbias[:])

                for mk, msk, sc, gb_t in ((mkb, maskb, scb, gbb),
                                          (mkm, maskm, scm, gbm)):
                    nc.gpsimd.tensor_mul(out=mk[:, g0:g1], in0=msk[:, g0:g1],
                                         in1=lnp32[:, g0:g1])
                    nc.vector.tensor_tensor_scan(
                        out=sc[:, g0:g1], data0=ones[:, g0:g1],
                        data1=mk[:, g0:g1], initial=0.0,
                        op0=Alu.mult, op1=Alu.add)
                    nc.gpsimd.ap_gather(
                        out_ap=gb_t[:, si, 1:].rearrange(
                            "p (t d) -> p t d", d=1),
                        in_ap=sc[:, g0:g1].rearrange(
                            "p (g d) -> p g d", d=1),
                        idxs_ap=git[:, si:si + 1],
                        channels=128, num_elems=g1 - g0, d=1, num_idxs=TPS)
                nc.vector.tensor_sub(
                    out=braw[:, 32 * si:32 * si + TPS],
                    in0=gbb[:, si, 1:], in1=gbb[:, si, :TPS])
                nc.vector.tensor_sub(
                    out=braw[:, 32 * si + TPS:32 * si + 32],
                    in0=gbm[:, si, 1:], in1=gbm[:, si, :TPS])

            # ---- postfix ----
            ex = pk.tile([128, 2 * nt], f32, tag="ex")
            nc.scalar.activation(out=ex[:], in_=braw[:], func=Act.Exp)
            exr = ex[:].rearrange("p (s h t) -> p s h t", s=NSEG, h=2)
            surv = pk.tile([128, nt], f32, tag="surv")
            sr = surv[:].rearrange("p (s t) -> p s t", s=NSEG)
            a0 = auxt[:][:, :, 0].rearrange("p (s t) -> p s t", s=NSEG)
            a1 = auxt[:][:, :, 1].rearrange("p (s t) -> p s t", s=NSEG)
            a2 = auxt[:][:, :, 2].rearrange("p (s t) -> p s t", s=NSEG)
            nc.vector.tensor_mul(out=sr, in0=exr[:, :, 0, :], in1=a0)
            nc.sync.dma_start(out=so_v, in_=surv[:])
            rate = pk.tile([128, nt], f32, tag="rate")
            rr = rate[:].rearrange("p (s t) -> p s t", s=NSEG)
            nc.vector.tensor_mul(out=rr, in0=exr[:, :, 1, :], in1=a1)
            nc.vector.tensor_add(out=rr, in0=rr, in1=a2)
            nc.scalar.dma_start(out=ro_v, in_=rate[:])
    nc.finalize()
    return nc


def host_aux(x, bid_info, widths):
    n = x.shape[0]
    mp = bid_info[:, 0].astype(np.int64)
    bid = bid_info[:, 1].astype(np.int64)
    rows = np.arange(n)
    straddle = {}
    for col, k in ((0, bid), (1, mp)):
        base = k - (k & (L - 1))
        s = np.ones(n, dtype=np.float32)
        for j in range(L - 1):
            idx = base + j
            take = x[rows, np.minimum(idx, S - 1)]
            s *= np.where(idx < k, take, np.float32(1.0))
        straddle[col] = s
    xmp = x[rows, mp]
    aux = np.zeros((n, 4), dtype=np.float16)
    aux[:, 0] = straddle[0].astype(np.float16)
    aux[:, 1] = (straddle[1] * (1.0 - xmp) * (mp != 0)).astype(np.float16)
    aux[:, 2] = (np.float32(EPS) * (mp == 0)).astype(np.float16)

    nb = np.array([w // L for w in widths], dtype=np.int64)
    o32 = np.concatenate([[0], np.cumsum(nb)])
    tile_of = np.tile(np.arange(NT), 128)
    base_blk = o32[tile_of]
    thr = np.empty((n, 2), dtype=np.float16)
    thr[:, 0] = (base_blk + bid // L).astype(np.float16)
    thr[:, 1] = (base_blk + mp // L).astype(np.float16)
    return aux, thr


def plan(bid_info):
    w = np.maximum(np.maximum(bid_info[:, 0], bid_info[:, 1]), 1)
    order = np.argsort(-w, kind="stable")
    perm = np.empty(B, dtype=np.int64)
    for c in range(N_CORES):
        perm[c * ROWS:(c + 1) * ROWS] = order[c::N_CORES]
    j = np.arange(ROWS)
    p, t = j // NT, j % NT
    src_slot = t * 128 + p
    ws = w[order]
    w_max = np.zeros(NT, dtype=np.int64)
    for t_i in range(NT):
        w_max[t_i] = int(ws[t_i * 1024:(t_i + 1) * 1024].max())
    widths = np.minimum(np.maximum(((w_max + L - 1) // L) * L, L), S)
    return perm, src_slot, tuple(int(v) for v in widths)


def pack_x(xc16, widths):
    """Per segment: blocks [128, NBs, 32] -> planes [128, 32, NBs], then
    chunk_a = planes 0-7 & 16-23, chunk_b = planes 8-15 & 24-31."""
    nb, o32, segw = seg_geometry(widths)
    seg_off = np.concatenate([[0], np.cumsum([w * L for w in segw])]).astype(int)
    Xr = xc16.reshape(128, NT, S)
    xpk = np.empty((128, int(seg_off[-1])), dtype=np.float16)
    for si in range(NSEG):
        t0, t1 = si * TPS, (si + 1) * TPS
        blocks = [Xr[:, t, :widths[t]].reshape(128, widths[t] // L, L)
                  for t in range(t0, t1)]
        blk = np.concatenate(blocks, axis=1)        # [128, NBs, 32]
        pl = blk.transpose(0, 2, 1)                 # [128, L planes, NBs]
        order = np.concatenate([np.arange(0, 16), np.arange(32, 48),
                                np.arange(16, 32), np.arange(48, 64)])
        xpk[:, seg_off[si]:seg_off[si + 1]] = pl[:, order, :].reshape(
            128, segw[si] * L)
    return xpk


def make_gidx(widths):
    nb = np.array([w // L for w in widths], dtype=np.int64)
    o32 = np.concatenate([[0], np.cumsum(nb)])
    gidx = np.zeros((128, NSEG), dtype=np.int16)
    for s in range(NSEG):
        ts0 = s * TPS
        rel_ends = (o32[ts0 + 1:ts0 + TPS + 1] - 1 - o32[ts0]).astype(np.int16)
        for p in range(128):
            gidx[p, s] = rel_ends[p % 16]
    return gidx


_NC_CACHE = {}


def _get_nc(widths):
    if widths not in _NC_CACHE:
        _NC_CACHE.clear()
        _NC_CACHE[widths] = build_bass(list(widths))
    return _NC_CACHE[widths]


def kernel(x, bid_info):
    x = np.ascontiguousarray(np.asarray(x, dtype=np.float32))
    bid_info = np.ascontiguousarray(np.asarray(bid_info, dtype=np.int32))
    assert x.shape == (B, S) and bid_info.shape == (B, 2)

    perm, src_slot, widths = plan(bid_info)
    nc = _get_nc(widths)
    gidx = make_gidx(widths)

    in_maps, core_rows = [], []
    for c in range(N_CORES):
        rows_c = perm[c * ROWS:(c + 1) * ROWS][src_slot]
        core_rows.append(rows_c)
        xc = x[rows_c]
        bc = bid_info[rows_c]
        aux, thr = host_aux(xc, bc, widths)
        in_maps.append({"xpk": pack_x(xc.astype(np.float16), widths),
                        "aux16": aux, "thr16": thr, "gidx": gidx})
    res = run_bass_kernel_spmd(nc, in_maps, core_ids=list(range(N_CORES)))
    survival = np.empty((B, 1), dtype=np.float32)
    rate_last = np.empty((B, 1), dtype=np.float32)
    for c in range(N_CORES):
        survival[core_rows[c]] = res.results[c]["survival"]
        rate_last[core_rows[c]] = res.results[c]["rate_last"]
    return survival, rate_last


# revision 3
# speedup vs baseline: 1.0612x; 1.0612x over previous
"""Trainium2 Bass kernel v5 for nn_BidPrefix (segment_reduce).

Per row r (B=65536, S=512): cp[k] = prod(x[r, 0:k]).  Outputs:
    survival = cp[bid]
    rate     = cp[mp] - cp[mp+1] = cp[mp] * (1 - x[mp])   (EPS when mp == 0)

v5: segment-granular pipeline (4 segments x 16 tiles), L = 32 blocks.
  - Host sorts rows by w = max(bid, mp) descending; per-tile widths W_t
    (multiple of 32).  Each segment's x is packed in a 32-plane halving
    layout, split into two DMA chunks with INTERLEAVED planes
    (a = planes 0-7+16-23, b = planes 8-15+24-31) so s1 runs per chunk:
      s1_a = a_lo*a_hi, s1_b = b_lo*b_hi -> b1;  s2..s5 halve b1 -> p32.
    6 fp16 2x TensorTensor per segment on DVE.
  - Act: ln(p32 + 1e-38) per segment -> lnp32 fp16 [128, tot32].
  - Prefix selection without per-tile ops:
      thrG[p,t] = o32_t + floor(k/32)   (host, fp16, global block coords)
      thrGB     = thrG bcast over blocks (Pool grouped stride-0 copies)
      iog       = global block iota      (Pool iota + copy)
      mask      = (thrGB > iog)          (DVE fp16 2x)
      mk        = mask * lnp32           (Pool mult, per segment)
      sc        = segment cumsum         (DVE tensor_tensor_scan, f32 out)
      g         = sc at static tile-end columns (Pool ap_gather)
      braw[t]   = g[t] - g[t-1]          (DVE sub per segment)
  - Act: exp on braw; DVE postfix with host-fused straddle scalars.
  - Segment processing order [2, 0, 1, 3]: cheap segment first (fast
    pipeline start) and cheapest last (short tail).
Straddle products over the <=31 leading elements of the boundary block and
x[mp] ship as fp16 aux scalars computed on host during sharding.
"""

import numpy as np

import concourse.bacc as bacc
from concourse.bass import AP
from concourse.hw_specs import get_activation_tables
import concourse.mybir as mybir
from concourse.tile import TileContext
from concourse.bass_utils import run_bass_kernel_spmd

f32 = mybir.dt.float32
f16 = mybir.dt.float16
i32 = mybir.dt.int32
i16 = mybir.dt.int16
Alu = mybir.AluOpType
Act = mybir.ActivationFunctionType

N_CORES = 8
B, S = 65536, 512
ROWS = B // N_CORES          # 8192 rows per core
NT = ROWS // 128             # 64 tiles of 128 rows
L = 64                       # block size
NSEG = 4
TPS = NT // NSEG             # 16 tiles per segment
SEG_ORDER = [2, 0, 1, 3]
EPS = 1e-7


def seg_geometry(widths):
    nb = [w // L for w in widths]
    o32 = np.concatenate([[0], np.cumsum(nb)]).astype(int)
    segw = [int(o32[(s + 1) * TPS] - o32[s * TPS]) for s in range(NSEG)]
    return nb, o32, segw


def build_bass(widths):
    nt = NT
    assert len(widths) == nt
    nb, o32, segw = seg_geometry(widths)
    tot32 = int(o32[-1])
    P = tot32 * L
    nc = bacc.Bacc()

    xpk = nc.dram_tensor("xpk", [128, P], f16, kind="ExternalInput")
    aux16 = nc.dram_tensor("aux16", [ROWS, 4], f16, kind="ExternalInput")
    thr16 = nc.dram_tensor("thr16", [ROWS, 2], f16, kind="ExternalInput")
    gidx = nc.dram_tensor("gidx", [128, NSEG], i16, kind="ExternalInput")
    surv_out = nc.dram_tensor("survival", [ROWS, 1], f32, kind="ExternalOutput")
    rate_out = nc.dram_tensor("rate_last", [ROWS, 1], f32, kind="ExternalOutput")

    aux_v = aux16.rearrange("(p t) c -> p t c", t=nt)
    thr_v = thr16.rearrange("(p t) c -> p t c", t=nt)
    so_v = surv_out.rearrange("(p t) c -> p (t c)", t=nt)
    ro_v = rate_out.rearrange("(p t) c -> p (t c)", t=nt)

    # xpk element offsets: segments in tile order; each segment is
    # [chunk_a | chunk_b], chunk = 16 planes * NBs elements
    seg_off = np.concatenate([[0], np.cumsum([w * L for w in segw])]).astype(int)
    sb_max = max(segw)

    groups = []
    t = 0
    while t < nt:
        t2 = t
        while t2 < nt and nb[t2] == nb[t]:
            t2 += 1
        groups.append((t, t2))
        t = t2

    with TileContext(nc) as tc:
        with (
            tc.tile_pool(name="xp", bufs=4) as xpool,
            tc.tile_pool(name="c1", bufs=2) as c1pool,
            tc.tile_pool(name="c2", bufs=2) as c2pool,
            tc.tile_pool(name="pk", bufs=1) as pk,
        ):
            # thr first: it gates mask prep and must beat x to the DMA queue
            thrt = pk.tile([128, nt, 2], f16, tag="thrt")
            nc.sync.dma_start(out=thrt[:], in_=thr_v)
            auxt = pk.tile([128, nt, 4], f16, tag="auxt")
            nc.scalar.dma_start(out=auxt[:], in_=aux_v)
            git = pk.tile([128, NSEG], i16, tag="git")
            nc.scalar.dma_start(out=git[:], in_=gidx[:, :])

            names = list(get_activation_tables(nc.m.arch).keys())
            nc.scalar.add_instruction(mybir.InstLoadActFuncSet(
                name=nc.get_next_instruction_name(),
                act_func_set_id=names.index("natural_log_exp_and_others"),
                ins=[], outs=[]))

            # Pool prep: iota, iog, ones, thrGB broadcast
            it = pk.tile([128, tot32], i32, tag="it")
            nc.gpsimd.iota(it[:], pattern=[[1, tot32]], base=0,
                           channel_multiplier=0)
            iog = pk.tile([128, tot32], f16, tag="iog")
            nc.vector.tensor_copy(out=iog[:], in_=it[:])
            ones = pk.tile([128, tot32], f16, tag="ones")
            nc.gpsimd.memset(ones[:], 1.0)

            thrgb = pk.tile([128, 2, tot32], f16, tag="thrgb")
            for (ga, gb) in groups:
                nbg = nb[ga]
                gsz = gb - ga
                src = thrt[:, ga:gb, :]
                bsrc = AP(src.tensor, src.offset,
                          [src.ap[0], [1, 2], [2, gsz], [0, nbg]])
                dst = thrgb[:, :, int(o32[ga]):int(o32[gb])]
                bdst = AP(dst.tensor, dst.offset,
                          [dst.ap[0], [tot32, 2], [nbg, gsz], [1, nbg]])
                nc.vector.tensor_copy(out=bdst, in_=bsrc)

            # masks on DVE (runs in the pre-first-chunk window)
            maskb = pk.tile([128, tot32], f16, tag="maskb")
            nc.vector.tensor_tensor(out=maskb[:], in0=thrgb[:, 0, :],
                                    in1=iog[:], op=Alu.is_gt)
            maskm = pk.tile([128, tot32], f16, tag="maskm")
            nc.vector.tensor_tensor(out=maskm[:], in0=thrgb[:, 1, :],
                                    in1=iog[:], op=Alu.is_gt)

            lnbias = pk.tile([128, 1], f32, tag="lnbias")
            nc.vector.memset(lnbias[:], 1e-38)

            p32 = pk.tile([128, tot32], f16, tag="p32")
            lnp32 = pk.tile([128, tot32], f16, tag="lnp32")
            mkb = pk.tile([128, tot32], f16, tag="mkb")
            mkm = pk.tile([128, tot32], f16, tag="mkm")
            scb = pk.tile([128, tot32], f32, tag="scb")
            scm = pk.tile([128, tot32], f32, tag="scm")
            braw = pk.tile([128, 2 * nt], f32, tag="braw")
            gbb = pk.tile([128, NSEG, TPS + 1], f32, tag="gbb")
            gbm = pk.tile([128, NSEG, TPS + 1], f32, tag="gbm")
            nc.vector.memset(gbb[:, :, 0], 0.0)
            nc.vector.memset(gbm[:, :, 0], 0.0)

            for si in SEG_ORDER:
                NBs = segw[si]
                E = int(seg_off[si])          # xpk element base of segment
                g0 = int(o32[si * TPS])
                g1 = int(o32[(si + 1) * TPS])
                H = (L // 4) * NBs            # half-chunk width

                xa = xpool.tile([128, sb_max * (L // 2)], f16, tag="xa")
                nc.sync.dma_start(out=xa[:, :2 * H], in_=xpk[:, E:E + 2 * H])
                xb_ = xpool.tile([128, sb_max * (L // 2)], f16, tag="xb")
                nc.sync.dma_start(out=xb_[:, :2 * H],
                                  in_=xpk[:, E + 2 * H:E + 4 * H])

                b1 = c1pool.tile([128, sb_max * 32], f16, tag="b1")
                nc.vector.tensor_tensor(out=b1[:, :H], in0=xa[:, :H],
                                        in1=xa[:, H:2 * H], op=Alu.mult)
                nc.vector.tensor_tensor(out=b1[:, H:2 * H], in0=xb_[:, :H],
                                        in1=xb_[:, H:2 * H], op=Alu.mult)
                b2 = c2pool.tile([128, sb_max * 16], f16, tag="b2")
                nc.vector.tensor_tensor(out=b2[:, :H], in0=b1[:, :H],
                                        in1=b1[:, H:2 * H], op=Alu.mult)
                b3 = c2pool.tile([128, sb_max * 8], f16, tag="b3")
                nc.vector.tensor_tensor(out=b3[:, :H // 2], in0=b2[:, :H // 2],
                                        in1=b2[:, H // 2:H], op=Alu.mult)
                b4 = c2pool.tile([128, sb_max * 4], f16, tag="b4")
                nc.vector.tensor_tensor(out=b4[:, :H // 4], in0=b3[:, :H // 4],
                                        in1=b3[:, H // 4:H // 2], op=Alu.mult)
                b5 = c2pool.tile([128, sb_max * 2], f16, tag="b5")
                nc.vector.tensor_tensor(out=b5[:, :H // 8], in0=b4[:, :H // 8],
                                        in1=b4[:, H // 8:H // 4], op=Alu.mult)
                nc.vector.tensor_tensor(out=p32[:, g0:g1], in0=b5[:, :H // 16],
                                        in1=b5[:, H // 16:H // 8], op=Alu.mult)
                nc.scalar.activation(out=lnp32[:, g0:g1], in_=p32[:, g0:g1],
                                     func=Act.Ln, bias=lnbias[:])

                for mk, msk, sc, gb_t in ((mkb, maskb, scb, gbb),
                                          (mkm, maskm, scm, gbm)):
                    nc.gpsimd.tensor_mul(out=mk[:, g0:g1], in0=msk[:, g0:g1],
                                         in1=lnp32[:, g0:g1])
                    nc.vector.tensor_tensor_scan(
                        out=sc[:, g0:g1], data0=ones[:, g0:g1],
                        data1=mk[:, g0:g1], initial=0.0,
                        op0=Alu.mult, op1=Alu.add)
                    nc.gpsimd.ap_gather(
                        out_ap=gb_t[:, si, 1:].rearrange(
                            "p (t d) -> p t d", d=1),
                        in_ap=sc[:, g0:g1].rearrange(
                            "p (g d) -> p g d", d=1),
                        idxs_ap=git[:, si:si + 1],
                        channels=128, num_elems=g1 - g0, d=1, num_idxs=TPS)
                nc.vector.tensor_sub(
                    out=braw[:, 32 * si:32 * si + TPS],
                    in0=gbb[:, si, 1:], in1=gbb[:, si, :TPS])
                nc.vector.tensor_sub(
                    out=braw[:, 32 * si + TPS:32 * si + 32],
                    in0=gbm[:, si, 1:], in1=gbm[:, si, :TPS])

            # ---- postfix ----
            ex = pk.tile([128, 2 * nt], f32, tag="ex")
            nc.scalar.activation(out=ex[:], in_=braw[:], func=Act.Exp)
            exr = ex[:].rearrange("p (s h t) -> p s h t", s=NSEG, h=2)
            surv = pk.tile([128, nt], f32, tag="surv")
            sr = surv[:].rearrange("p (s t) -> p s t", s=NSEG)
            a0 = auxt[:][:, :, 0].rearrange("p (s t) -> p s t", s=NSEG)
            a1 = auxt[:][:, :, 1].rearrange("p (s t) -> p s t", s=NSEG)
            a2 = auxt[:][:, :, 2].rearrange("p (s t) -> p s t", s=NSEG)
            nc.vector.tensor_mul(out=sr, in0=exr[:, :, 0, :], in1=a0)
            nc.sync.dma_start(out=so_v, in_=surv[:])
            rate = pk.tile([128, nt], f32, tag="rate")
            rr = rate[:].rearrange("p (s t) -> p s t", s=NSEG)
            nc.vector.tensor_mul(out=rr, in0=exr[:, :, 1, :], in1=a1)
            nc.vector.tensor_add(out=rr, in0=rr, in1=a2)
            nc.scalar.dma_start(out=ro_v, in_=rate[:])
    nc.finalize()
    return nc


def host_aux(x, bid_info, widths):
    n = x.shape[0]
    mp = bid_info[:, 0].astype(np.int64)
    bid = bid_info[:, 1].astype(np.int64)
    rows = np.arange(n)
    straddle = {}
    for col, k in ((0, bid), (1, mp)):
        base = k - (k & (L - 1))
        s = np.ones(n, dtype=np.float32)
        for j in range(L - 1):
            idx = base + j
            take = x[rows, np.minimum(idx, S - 1)]
            s *= np.where(idx < k, take, np.float32(1.0))
        straddle[col] = s
    xmp = x[rows, mp]
    aux = np.zeros((n, 4), dtype=np.float16)
    aux[:, 0] = straddle[0].astype(np.float16)
    aux[:, 1] = (straddle[1] * (1.0 - xmp) * (mp != 0)).astype(np.float16)
    aux[:, 2] = (np.float32(EPS) * (mp == 0)).astype(np.float16)

    nb = np.array([w // L for w in widths], dtype=np.int64)
    o32 = np.concatenate([[0], np.cumsum(nb)])
    tile_of = np.tile(np.arange(NT), 128)
    base_blk = o32[tile_of]
    thr = np.empty((n, 2), dtype=np.float16)
    thr[:, 0] = (base_blk + bid // L).astype(np.float16)
    thr[:, 1] = (base_blk + mp // L).astype(np.float16)
    return aux, thr


def plan(bid_info):
    w = np.maximum(np.maximum(bid_info[:, 0], bid_info[:, 1]), 1)
    order = np.argsort(-w, kind="stable")
    perm = np.empty(B, dtype=np.int64)
    for c in range(N_CORES):
        perm[c * ROWS:(c + 1) * ROWS] = order[c::N_CORES]
    j = np.arange(ROWS)
    p, t = j // NT, j % NT
    src_slot = t * 128 + p
    ws = w[order]
    w_max = np.zeros(NT, dtype=np.int64)
    for t_i in range(NT):
        w_max[t_i] = int(ws[t_i * 1024:(t_i + 1) * 1024].max())
    # The partial last block is handled by the host straddle (elements
    # [L*floor(k/L), k)), so the device only needs floor(w/L) full blocks.
    widths = np.minimum(np.maximum((w_max // L) * L, L), S)
    return perm, src_slot, tuple(int(v) for v in widths)


def pack_x(xc16, widths):
    """Per segment: blocks [128, NBs, 32] -> planes [128, 32, NBs], then
    chunk_a = planes 0-7 & 16-23, chunk_b = planes 8-15 & 24-31."""
    nb, o32, segw = seg_geometry(widths)
    seg_off = np.concatenate([[0], np.cumsum([w * L for w in segw])]).astype(int)
    Xr = xc16.reshape(128, NT, S)
    xpk = np.empty((128, int(seg_off[-1])), dtype=np.float16)
    for si in range(NSEG):
        t0, t1 = si * TPS, (si + 1) * TPS
        blocks = [Xr[:, t, :widths[t]].reshape(128, widths[t] // L, L)
                  for t in range(t0, t1)]
        blk = np.concatenate(blocks, axis=1)        # [128, NBs, 32]
        pl = blk.transpose(0, 2, 1)                 # [128, L planes, NBs]
        order = np.concatenate([np.arange(0, 16), np.arange(32, 48),
                                np.arange(16, 32), np.arange(48, 64)])
        xpk[:, seg_off[si]:seg_off[si + 1]] = pl[:, order, :].reshape(
            128, segw[si] * L)
    return xpk


def make_gidx(widths):
    nb = np.array([w // L for w in widths], dtype=np.int64)
    o32 = np.concatenate([[0], np.cumsum(nb)])
    gidx = np.zeros((128, NSEG), dtype=np.int16)
    for s in range(NSEG):
        ts0 = s * TPS
        rel_ends = (o32[ts0 + 1:ts0 + TPS + 1] - 1 - o32[ts0]).astype(np.int16)
        for p in range(128):
            gidx[p, s] = rel_ends[p % 16]
    return gidx


_NC_CACHE = {}


def _get_nc(widths):
    if widths not in _NC_CACHE:
        _NC_CACHE.clear()
        _NC_CACHE[widths] = build_bass(list(widths))
    return _NC_CACHE[widths]


def kernel(x, bid_info):
    x = np.ascontiguousarray(np.asarray(x, dtype=np.float32))
    bid_info = np.ascontiguousarray(np.asarray(bid_info, dtype=np.int32))
    assert x.shape == (B, S) and bid_info.shape == (B, 2)

    perm, src_slot, widths = plan(bid_info)
    nc = _get_nc(widths)
    gidx = make_gidx(widths)

    in_maps, core_rows = [], []
    for c in range(N_CORES):
        rows_c = perm[c * ROWS:(c + 1) * ROWS][src_slot]
        core_rows.append(rows_c)
        xc = x[rows_c]
        bc = bid_info[rows_c]
        aux, thr = host_aux(xc, bc, widths)
        in_maps.append({"xpk": pack_x(xc.astype(np.float16), widths),
                        "aux16": aux, "thr16": thr, "gidx": gidx})
    res = run_bass_kernel_spmd(nc, in_maps, core_ids=list(range(N_CORES)))
    survival = np.empty((B, 1), dtype=np.float32)
    rate_last = np.empty((B, 1), dtype=np.float32)
    for c in range(N_CORES):
        survival[core_rows[c]] = res.results[c]["survival"]
        rate_last[core_rows[c]] = res.results[c]["rate_last"]
    return survival, rate_last
